# revision 1
# baseline (speedup 1.0000x reference)
"""Trainium2 Bass kernel for nn_Attention_RoPE (LN -> QKV -> RoPE -> attention -> out-proj).

Sharding: 8 cores = 4 batches x 2 head-groups (8 heads each).
Each core computes a partial out-projection [S, D] for its (batch, head-group);
host sums the two partials per batch and adds b_out.

Per-core pipeline (single Bass program, SPMD over 8 cores):
  phase 1+2 (fused, per 128-row seq tile):
    LN (bn_stats/bn_aggr, DVE) -> PE transpose to xnT -> QKV matmuls (bf16)
    -> RoPE on q,k (DVE, from PSUM) -> PE transpose to qT/kT [feat, seq]
    -> v copied to SBUF with an appended ones column (for softmax denominators)
  phase 3 (per head-pair, per 512-query block):
    S^T = K @ Q^T via row-packed matmuls (head0 rows 0:64, head1 rows 64:128)
    exp on ScalarE over [128, 1024] tiles (pair-packed; the throughput bottleneck)
    PV via lhsT=exp(S^T), rhs=[v | ones]  -> denominators for free
    scale by 1/sum, PE transpose into attn_outT [feat, seq]
  phase 4: out-projection, DMA partial result
"""

import numpy as np
import sys

sys.path.insert(0, "/opt/trn_rl_repo")

import ml_dtypes

import concourse.bass as bass
from concourse import bacc
import concourse.mybir as mybir
import concourse.tile as tile
from concourse.masks import make_identity
from concourse.bass_utils import run_bass_kernel_spmd

# Problem constants (hardcoded per contract)
B, S, D = 4, 2048, 1024
H, DH = 16, 64
HG = 2              # head groups (tensor-parallel dim)
NH = H // HG        # heads per core = 8
IN = NH * DH        # per-core inner dim = 512
P = 128
NT = S // P         # 16 seq tiles
NCK = D // P        # 8 contraction chunks
NPAIR = NH // 2     # 4 head pairs
QB = 512            # query block in phase 3
EPS = 1e-5
BASE = 10000.0

F32 = mybir.dt.float32
BF16 = mybir.dt.bfloat16

_CACHE = {}


def _build_nc():
    nc = bacc.Bacc(None, target_bir_lowering=False, debug=False)

    x_d = nc.declare_dram_parameter("x", [S, D], F32, isOutput=False)
    wq_d = nc.declare_dram_parameter("wq", [D, IN], BF16, isOutput=False)
    wk_d = nc.declare_dram_parameter("wk", [D, IN], BF16, isOutput=False)
    wv_d = nc.declare_dram_parameter("wv", [D, IN], BF16, isOutput=False)
    wo_d = nc.declare_dram_parameter("wo", [IN, D], F32, isOutput=False)
    cos_d = nc.declare_dram_parameter("cos_rep", [S, NH * 32], F32, isOutput=False)
    sin_d = nc.declare_dram_parameter("sin_rep", [S, NH * 32], F32, isOutput=False)
    out_d = nc.declare_dram_parameter("out", [S, D], F32, isOutput=True)

    with tile.TileContext(nc) as tc:
        with tc.tile_pool(name="persist", bufs=1) as pers:
            ident = pers.tile([P, P], BF16)
            make_identity(nc, ident)
            eps_t = pers.tile([P, 1], F32)
            nc.vector.memset(eps_t, EPS)

            # weights resident in SBUF
            wq_s = pers.tile([P, NCK, IN], BF16, tag="wq")
            wk_s = pers.tile([P, NCK, IN], BF16, tag="wk")
            wv_s = pers.tile([P, NCK, IN], BF16, tag="wv")
            for w_s, w_d in ((wq_s, wq_d), (wk_s, wk_d), (wv_s, wv_d)):
                nc.sync.dma_start(
                    out=w_s, in_=w_d.rearrange("(c p) n -> p c n", p=P)
                )
            wo_s = pers.tile([P, 4, D], F32, tag="wo")
            nc.sync.dma_start(out=wo_s, in_=wo_d.rearrange("(c p) n -> p c n", p=P))

            # persistent activations
            qT = [pers.tile([P, S], BF16, tag=f"qT{i}", name=f"qT{i}") for i in range(NPAIR)]
            kT = [pers.tile([P, S], BF16, tag=f"kT{i}", name=f"kT{i}") for i in range(NPAIR)]
            # v with ones column appended per head: [kpos, head, 65]
            v_aug = [pers.tile([P, NH, DH + 1], F32, tag=f"v{i}", name=f"v{i}") for i in range(NT)]
            attnT = [pers.tile([P, S], F32, tag=f"aT{i}", name=f"aT{i}") for i in range(NPAIR)]

            # ---------------- phase 1 + 2 ----------------
            with tc.tile_pool(name="ph12", bufs=4) as tp, \
                 tc.tile_pool(name="ph12s", bufs=6) as sp, \
                 tc.tile_pool(name="ps_tr", bufs=2, space="PSUM") as ptr, \
                 tc.tile_pool(name="ps_proj", bufs=2, space="PSUM") as ppr:
                for s in range(NT):
                    r0 = s * P
                    x_s = tp.tile([P, D], F32, tag="x")
                    nc.gpsimd.dma_start(out=x_s, in_=x_d[r0 : r0 + P, :])

                    stats = sp.tile([P, 2, 6], F32, tag="stats")
                    for i in range(2):
                        nc.vector.bn_stats(
                            out=stats[:, i, :], in_=x_s[:, i * 512 : (i + 1) * 512]
                        )
                    mv = sp.tile([P, 2], F32, tag="mv")
                    nc.vector.bn_aggr(out=mv, in_=stats)
                    std = sp.tile([P, 1], F32, tag="std")
                    nc.scalar.activation(
                        out=std, in_=mv[:, 1:2],
                        func=mybir.ActivationFunctionType.Sqrt, bias=eps_t,
                    )
                    rstd = sp.tile([P, 1], F32, tag="rstd")
                    nc.vector.reciprocal(out=rstd, in_=std)

                    xn_s = tp.tile([P, D], BF16, tag="xn")
                    nc.vector.tensor_scalar(
                        out=xn_s, in0=x_s, scalar1=mv[:, 0:1], scalar2=rstd,
                        op0=mybir.AluOpType.subtract, op1=mybir.AluOpType.mult,
                    )

                    # transpose xn -> xnT blocks [dim, seq]
                    xnT_s = tp.tile([P, NCK, P], BF16, tag="xnT")
                    for c in range(NCK):
                        pt = ptr.tile([P, P], BF16, tag="tr")
                        nc.tensor.transpose(pt, xn_s[:, c * P : (c + 1) * P], ident)
                        nc.vector.tensor_copy(out=xnT_s[:, c, :], in_=pt)

                    # projections: q, k, v  [128 seq, 512 feat]
                    ps_q = ppr.tile([P, IN], F32, tag="pq")
                    ps_k = ppr.tile([P, IN], F32, tag="pk")
                    ps_v = ppr.tile([P, IN], F32, tag="pv")
                    for ps, w_s in ((ps_q, wq_s), (ps_k, wk_s), (ps_v, wv_s)):
                        for c in range(NCK):
                            nc.tensor.matmul(
                                ps, lhsT=xnT_s[:, c, :], rhs=w_s[:, c, :],
                                start=(c == 0), stop=(c == NCK - 1),
                            )

                    # v -> SBUF with ones column
                    v_s = v_aug[s]
                    nc.vector.tensor_copy(
                        out=v_s[:, :, 0:DH],
                        in_=ps_v.rearrange("p (h d) -> p h d", h=NH),
                    )
                    nc.vector.memset(v_s[:, :, DH : DH + 1], 1.0)

                    # RoPE on q, k (into bf16 rot tiles)
                    cos_s = sp.tile([P, NH, 32], F32, tag="cos")
                    sin_s = sp.tile([P, NH, 32], F32, tag="sin")
                    nc.gpsimd.dma_start(
                        out=cos_s, in_=cos_d[r0 : r0 + P, :].rearrange("p (h d) -> p h d", h=NH)
                    )
                    nc.gpsimd.dma_start(
                        out=sin_s, in_=sin_d[r0 : r0 + P, :].rearrange("p (h d) -> p h d", h=NH)
                    )
                    for name, ps in (("q", ps_q), ("k", ps_k)):
                        p3 = ps.rearrange("p (h d) -> p h d", h=NH)
                        x1, x2 = p3[:, :, 0:32], p3[:, :, 32:64]
                        rot = tp.tile([P, NH, DH], BF16, tag=f"rot{name}")
                        t1 = sp.tile([P, NH, 32], F32, tag="t1")
                        t2 = sp.tile([P, NH, 32], F32, tag="t2")
                        nc.vector.tensor_mul(t1, x1, cos_s)
                        nc.vector.tensor_mul(t2, x2, sin_s)
                        nc.vector.tensor_sub(rot[:, :, 0:32], t1, t2)
                        nc.vector.tensor_mul(t1, x1, sin_s)
                        nc.vector.tensor_mul(t2, x2, cos_s)
                        nc.vector.tensor_add(rot[:, :, 32:64], t1, t2)
                        # transpose rot -> qT/kT [feat, seq] (2 heads per 128-block)
                        dstl = qT if name == "q" else kT
                        rflat = rot.rearrange("p h d -> p (h d)")
                        for fg in range(NPAIR):
                            pt = ptr.tile([P, P], BF16, tag="tr")
                            nc.tensor.transpose(
                                pt, rflat[:, fg * P : (fg + 1) * P], ident
                            )
                            nc.vector.tensor_copy(
                                out=dstl[fg][:, r0 : r0 + P], in_=pt
                            )

            # ---------------- phase 3: attention ----------------
            scale = 1.0 / np.sqrt(DH)
            with tc.tile_pool(name="pt_pool", bufs=6) as ptp, \
                 tc.tile_pool(name="sc_pool", bufs=3) as scp, \
                 tc.tile_pool(name="sm_pool", bufs=8) as smp, \
                 tc.tile_pool(name="ps_st", bufs=2, space="PSUM") as pst, \
                 tc.tile_pool(name="ps_pv", bufs=1, space="PSUM") as ppv, \
                 tc.tile_pool(name="dram_sc", bufs=8, space="DRAM") as dpool:
                for pair in range(NPAIR):
                    for qb2 in range(2):
                        # two query blocks in flight: independent chains hide sem latency
                        pvTs = {}
                        for j in range(2):
                            for hh in range(2):
                                pvTs[(j, hh)] = ppv.tile(
                                    [DH + 1, QB], F32, tag=f"pvT{j}{hh}", name=f"pvT{j}{hh}"
                                )
                        for kb in range(NT):
                            for j in range(2):
                                q0 = (qb2 * 2 + j) * QB
                                ps_st = pst.tile([P, 2 * QB], F32, tag="st")
                                for hh in range(2):
                                    nc.tensor.matmul(
                                        ps_st[:, hh * QB : (hh + 1) * QB],
                                        lhsT=kT[pair][hh * 64 : (hh + 1) * 64, kb * P : (kb + 1) * P],
                                        rhs=qT[pair][hh * 64 : (hh + 1) * 64, q0 : q0 + QB],
                                        start=True, stop=True,
                                    )
                                pt_t = ptp.tile([P, 2 * QB], F32, tag="pt")
                                nc.scalar.activation(
                                    out=pt_t, in_=ps_st,
                                    func=mybir.ActivationFunctionType.Exp, scale=scale,
                                )
                                for hh in range(2):
                                    nc.tensor.matmul(
                                        pvTs[(j, hh)],
                                        lhsT=v_aug[kb][:, pair * 2 + hh, :],
                                        rhs=pt_t[:, hh * QB : (hh + 1) * QB],
                                        start=(kb == 0), stop=(kb == NT - 1),
                                    )
                        # epilogue: free PSUM accumulators via SBUF copy, then
                        # scale by 1/rowsum (row replicated via DRAM-bounce broadcast)
                        for j in range(2):
                            q0 = (qb2 * 2 + j) * QB
                            for hh in range(2):
                                pvsb = scp.tile([DH + 1, QB], F32, tag="pvsb")
                                nc.vector.tensor_copy(pvsb, pvTs[(j, hh)])
                                rec = smp.tile([1, QB], F32, tag="rec")
                                nc.vector.reciprocal(rec, pvsb[DH : DH + 1, :])
                                dsc = dpool.tile([1, QB], F32, tag="dsc")
                                nc.sync.dma_start(out=dsc, in_=rec)
                                rep = smp.tile([64, QB], F32, tag="repsb")
                                nc.sync.dma_start(
                                    out=rep,
                                    in_=bass.AP(tensor=dsc.tensor, offset=dsc.offset,
                                                ap=[[0, 64], list(dsc.ap[-1])]),
                                )
                                if hh == 0:
                                    nc.vector.tensor_mul(
                                        attnT[pair][0:64, q0 : q0 + QB],
                                        pvsb[0:DH, :], rep,
                                    )
                                else:
                                    sc_h = scp.tile([64, QB], F32, tag="sc")
                                    nc.vector.tensor_mul(sc_h, pvsb[0:DH, :], rep)
                                    nc.sync.dma_start(
                                        out=attnT[pair][64:128, q0 : q0 + QB], in_=sc_h
                                    )

            # ---------------- phase 4: out projection ----------------
            with tc.tile_pool(name="ps_out", bufs=4, space="PSUM") as pso, \
                 tc.tile_pool(name="sb_out", bufs=3) as sbo:
                for s in range(NT):
                    r0 = s * P
                    o_s = sbo.tile([P, D], F32, tag="osb")
                    for n in range(2):
                        ps_o = pso.tile([P, 512], F32, tag="out")
                        for c in range(4):
                            nc.tensor.matmul(
                                ps_o,
                                lhsT=attnT[c][:, r0 : r0 + P],
                                rhs=wo_s[:, c, n * 512 : (n + 1) * 512],
                                start=(c == 0), stop=(c == 3),
                            )
                        nc.vector.tensor_copy(
                            out=o_s[:, n * 512 : (n + 1) * 512], in_=ps_o
                        )
                    nc.scalar.dma_start(out=out_d[r0 : r0 + P, :], in_=o_s)
    nc.compile()
    return nc


def _rope_tables():
    inv = 1.0 / (BASE ** (np.arange(0, DH, 2, dtype=np.float32) / DH))
    t = np.arange(S, dtype=np.float32)
    freqs = t[:, None] * inv[None, :]  # [S, 32]
    cos_rep = np.tile(np.cos(freqs), (1, NH)).astype(np.float32)
    sin_rep = np.tile(np.sin(freqs), (1, NH)).astype(np.float32)
    return np.ascontiguousarray(cos_rep), np.ascontiguousarray(sin_rep)


def kernel(x, w_qkv, w_out, b_out, ln_gamma, ln_beta, _want_results=False, _trace=False):
    x = np.asarray(x, dtype=np.float32)
    w_qkv = np.asarray(w_qkv, dtype=np.float32)
    w_out = np.asarray(w_out, dtype=np.float32)
    b_out = np.asarray(b_out, dtype=np.float32)
    ln_gamma = np.asarray(ln_gamma, dtype=np.float32)
    ln_beta = np.asarray(ln_beta, dtype=np.float32)
    assert np.all(ln_beta == 0.0), "nonzero ln_beta not supported by this kernel"

    if "nc" not in _CACHE:
        _CACHE["nc"] = _build_nc()
    nc = _CACHE["nc"]

    wg = w_qkv * ln_gamma[:, None]  # fold gamma into the projection
    cos_rep, sin_rep = _rope_tables()
    bf = ml_dtypes.bfloat16

    in_maps = []
    for core in range(8):
        b, hg = core // HG, core % HG
        c0 = hg * IN
        in_maps.append({
            "x": np.ascontiguousarray(x[b]),
            "wq": np.ascontiguousarray(wg[:, c0 : c0 + IN]).astype(bf),
            "wk": np.ascontiguousarray(wg[:, D + c0 : D + c0 + IN]).astype(bf),
            "wv": np.ascontiguousarray(wg[:, 2 * D + c0 : 2 * D + c0 + IN]).astype(bf),
            "wo": np.ascontiguousarray(w_out[c0 : c0 + IN, :]),
            "cos_rep": cos_rep,
            "sin_rep": sin_rep,
        })

    res = run_bass_kernel_spmd(nc, in_maps, list(range(8)), trace=_trace)
    parts = [res.results[c]["out"] for c in range(8)]
    out = np.empty((B, S, D), dtype=np.float32)
    for b in range(B):
        out[b] = parts[2 * b] + parts[2 * b + 1] + b_out[None, :]
    if _want_results:
        return out, res
    return out


if __name__ == "__main__":
    rng = np.random.default_rng(0)
    inputs = {
        "x": rng.standard_normal((B, S, D), dtype=np.float32),
        "w_qkv": (rng.standard_normal((D, 3 * D), dtype=np.float32) * D ** -0.5),
        "w_out": (rng.standard_normal((D, D), dtype=np.float32) * D ** -0.5),
        "b_out": np.zeros(D, np.float32),
        "ln_gamma": np.ones(D, np.float32),
        "ln_beta": np.zeros(D, np.float32),
    }
    out = kernel(**inputs)
    print("ok", out.shape, out.dtype)



# revision 7
# speedup vs baseline: 2.0885x; 2.0885x over previous
"""Trainium2 Bass kernel for nn_Attention_RoPE (LN -> QKV -> RoPE -> attention -> out-proj).

Sharding: 8 cores = 4 batches x 2 head-groups (8 heads each).
Each core computes a partial out-projection [S, D] (fp16) for its
(batch, head-group); host sums the two partials per batch and adds b_out.

Per-core pipeline (single Bass program, SPMD over 8 cores), fp16 matmuls:
  LN pass (per 128-row seq tile): stats via ScalarE accum (Copy/Square +
    accum_out), normalize on GPSIMD -> fp16, xn -> xnT via DMA xbar
    transpose. All sqrts precede all exps (single act-table switch).
  proj pass (per tile): QKV matmuls (fp16), RoPE on q,k (DVE: two
    elementwise products against [cos|sin]/[sin|cos] tables broadcast
    per head + add/sub), one DMA xbar transpose -> qkT; v -> SBUF fp16
    with an appended ones column (softmax denominators ride the PV
    matmul). Attention chunks for (qb0, pairs 0/1) are interleaved
    between tiles to keep the PE ramped and start the exp stream early.
  attention chunk (qb, pair, kb): S^T = K@Q^T (two PE-quadrant matmuls,
    kpos on partitions, K=64) -> exp on ScalarE [128,1024] -> PV with
    moving dim = dh+1 (N=65, q on output partitions) accumulating over
    kb. The four 128-query sub-blocks share one PSUM bank per head:
    matmul start=True zeroes the whole bank, so only the first matmul
    per (pair, bank) round sets it (skip_group_check).
  epilogue per (qb,pair): DVE reciprocal + per-partition tensor_scalar
    scale -> attn_sb fp16; per qb: DMA xbar transpose -> attnT,
    out-projection, fp16 partial DMA'd out.
"""

import numpy as np
import sys

sys.path.insert(0, "/opt/trn_rl_repo")

import concourse.bass as bass
from concourse import bacc
import concourse.mybir as mybir
import concourse.tile as tile
from concourse.bass_utils import run_bass_kernel_spmd

# Problem constants (hardcoded per contract)
B, S, D = 4, 2048, 1024
H, DH = 16, 64
HG = 2              # head groups (tensor-parallel dim)
NH = H // HG        # heads per core = 8
IN = NH * DH        # per-core inner dim = 512
P = 128
NT = S // P         # 16 seq tiles
NCK = D // P        # 8 contraction chunks
NPAIR = NH // 2     # 4 head pairs
QB = 512            # query block
NQB = S // QB       # 4 query blocks
EPS = 1e-5
BASE = 10000.0

F32 = mybir.dt.float32
F16 = mybir.dt.float16

_CACHE = {}


def _build_nc():
    nc = bacc.Bacc(None, target_bir_lowering=False, debug=False)

    x_d = nc.declare_dram_parameter("x", [S, D], F32, isOutput=False)
    wq_d = nc.declare_dram_parameter("wq", [D, IN], F16, isOutput=False)
    wk_d = nc.declare_dram_parameter("wk", [D, IN], F16, isOutput=False)
    wv_d = nc.declare_dram_parameter("wv", [D, IN], F16, isOutput=False)
    wo_d = nc.declare_dram_parameter("wo", [IN, D], F16, isOutput=False)
    ta_d = nc.declare_dram_parameter("tab_a", [S, DH], F16, isOutput=False)
    tb_d = nc.declare_dram_parameter("tab_b", [S, DH], F16, isOutput=False)
    out_d = nc.declare_dram_parameter("out", [S, D], F16, isOutput=True)

    scale = 1.0 / np.sqrt(DH)

    with tile.TileContext(nc) as tc:
        with tc.tile_pool(name="persist", bufs=1) as pers:
            eps_t = pers.tile([P, 1], F32)
            nc.vector.memset(eps_t, EPS)

            # fp16 weights resident in SBUF; row (c, p) <-> contraction d = c*128+p
            wq_s = pers.tile([P, NCK, IN], F16, tag="wq")
            wk_s = pers.tile([P, NCK, IN], F16, tag="wk")
            wv_s = pers.tile([P, NCK, IN], F16, tag="wv")
            wo_s = pers.tile([P, 4, D], F16, tag="wo")
            # RoPE tables [seq tile, 64]: A = [cos|sin], B = [sin|cos]
            ta_s = pers.tile([P, NT, DH], F16, tag="ta")
            tb_s = pers.tile([P, NT, DH], F16, tag="tb")

            # persistent activations
            # qkT: g<4 -> q pair g; g>=4 -> k pair g-4; row p = hh*64 + d
            qkT = pers.tile([P, 2 * NPAIR, S], F16, tag="qkT")
            v_s = pers.tile([P, NT, NH, DH + 1], F16, tag="v")
            nc.vector.memset(v_s[:, :, :, DH : DH + 1], 1.0)
            xnT_all = pers.tile([P, NT, NCK, P], F16, tag="xnT")

            nc.sync.dma_start(out=ta_s, in_=ta_d.rearrange("(t p) d -> p t d", p=P))
            nc.sync.dma_start(out=tb_s, in_=tb_d.rearrange("(t p) d -> p t d", p=P))
            for w_s, w_d, ncw in ((wq_s, wq_d, NCK), (wk_s, wk_d, NCK),
                                  (wv_s, wv_d, NCK), (wo_s, wo_d, 4)):
                for c in range(ncw):
                    nc.sync.dma_start(out=w_s[:, c, :], in_=w_d[c * P : (c + 1) * P, :])

            with tc.tile_pool(name="pst", bufs=2, space="PSUM") as pst, \
                 tc.tile_pool(name="pvm", bufs=1, space="PSUM") as pvm, \
                 tc.tile_pool(name="ptp", bufs=3) as ptp, \
                 tc.tile_pool(name="sc3", bufs=4) as sc3, \
                 tc.tile_pool(name="attn", bufs=2) as atp, \
                 tc.tile_pool(name="attnT", bufs=2) as atpT:

                # PV accumulators (pv slot = 2 banks): [q, qs, d*72pad|den@64]
                pv_main = [pvm.tile([P, 4, 72], F32, tag=f"pvm{hh}", name=f"pvm{hh}")
                           for hh in range(2)]

                attn_sb = {}   # qb -> [q, qs, feat] fp16

                def chunk(qb, pair, kb, pv):
                    """scores + exp + PV(+denominator column) for one (qb, pair, kb)."""
                    q0 = qb * QB
                    k0 = kb * P
                    ps_st = pst.tile([P, 2 * QB], F32, tag="st", name="ps_st")
                    for hh in range(2):
                        f0 = hh * DH
                        nc.tensor.matmul(
                            ps_st[:, hh * QB : (hh + 1) * QB],
                            lhsT=qkT[f0 : f0 + DH, NPAIR + pair, k0 : k0 + P],
                            rhs=qkT[f0 : f0 + DH, pair, q0 : q0 + QB],
                            start=True, stop=True,
                        )
                    pt = ptp.tile([P, 2 * QB], F16, tag="pt", name="pt")
                    nc.scalar.activation(
                        out=pt, in_=ps_st,
                        func=mybir.ActivationFunctionType.Exp, scale=scale,
                    )
                    for hh in range(2):
                        for qs in range(4):
                            nc.tensor.matmul(
                                pv[hh][:, qs, 0 : DH + 1],
                                lhsT=pt[:, hh * QB + qs * P : hh * QB + (qs + 1) * P],
                                rhs=v_s[:, kb, 2 * pair + hh, :],
                                start=(kb == 0 and qs == 0),
                                stop=(kb == NT - 1 and qs == 3),
                                skip_group_check=True,
                            )

                def pair_epilogue(qb, pair, pv):
                    a_sb = attn_sb[qb]
                    for hh in range(2):
                        rec = sc3.tile([P, 4, 1], F32, tag="rec", name="rec")
                        nc.vector.reciprocal(rec, pv[hh][:, :, DH : DH + 1])
                        for qs in range(4):
                            nc.vector.tensor_scalar(
                                out=a_sb[:, qs, pair * P + hh * DH : pair * P + (hh + 1) * DH],
                                in0=pv[hh][:, qs, 0:DH],
                                scalar1=rec[:, qs, :], scalar2=None,
                                op0=mybir.AluOpType.mult,
                            )

                def new_attn(qb):
                    attn_sb[qb] = atp.tile([P, 4, IN], F16, tag="attn", name=f"attn{qb}")

                def qb_outproj(qb, pso, osb):
                    aT = atpT.tile([P, 4, QB], F16, tag="aT", name=f"aT{qb}")
                    a_sb = attn_sb[qb]
                    for qs in range(4):
                        nc.sync.dma_start_transpose(
                            aT[:, :, qs * P : (qs + 1) * P], a_sb[:, qs, :]
                        )
                    for qs in range(4):
                        o_sb = osb.tile([P, D], F16, tag="osb", name="o_sb")
                        for n in range(2):
                            ps_o = pso.tile([P, 512], F32, tag="po", name="ps_o")
                            for c in range(4):
                                nc.tensor.matmul(
                                    ps_o,
                                    lhsT=aT[:, c, qs * P : (qs + 1) * P],
                                    rhs=wo_s[:, c, n * 512 : (n + 1) * 512],
                                    start=(c == 0), stop=(c == 3),
                                )
                            nc.vector.tensor_copy(
                                out=o_sb[:, n * 512 : (n + 1) * 512], in_=ps_o
                            )
                        r0 = qb * QB + qs * P
                        nc.scalar.dma_start(out=out_d[r0 : r0 + P, :], in_=o_sb)

                # ============ epoch 1: LN pass, proj pass + qb0 interleave ====
                with tc.tile_pool(name="pve", bufs=1, space="PSUM") as pve, \
                     tc.tile_pool(name="xp", bufs=3) as xp, \
                     tc.tile_pool(name="scr", bufs=1) as scrp, \
                     tc.tile_pool(name="sc1", bufs=4) as sc1, \
                     tc.tile_pool(name="xnp", bufs=3) as xnp, \
                     tc.tile_pool(name="rotp", bufs=3) as rotp, \
                     tc.tile_pool(name="abp", bufs=2) as abp:

                    pv_extra = [pve.tile([P, 4, 72], F32, tag=f"pve{hh}", name=f"pve{hh}")
                                for hh in range(2)]
                    pv_of = {0: pv_main, 1: pv_extra, 2: pv_main, 3: pv_extra}

                    # ---- LN pass: all stats/sqrts before any exp ----
                    for t in range(NT):
                        r0 = t * P
                        x_s = xp.tile([P, D], F32, tag="x", name="x_s")
                        nc.gpsimd.dma_start(out=x_s, in_=x_d[r0 : r0 + P, :])

                        scr = scrp.tile([P, D], F16, tag="scr", name="scr")
                        s1 = sc1.tile([P, 1], F32, tag="s1", name="s1")
                        s2 = sc1.tile([P, 1], F32, tag="s2", name="s2")
                        nc.scalar.activation(
                            out=scr, in_=x_s,
                            func=mybir.ActivationFunctionType.Copy, accum_out=s1,
                        )
                        nc.scalar.activation(
                            out=scr, in_=x_s,
                            func=mybir.ActivationFunctionType.Square, accum_out=s2,
                        )
                        mean = sc1.tile([P, 1], F32, tag="mean", name="mean")
                        nc.vector.tensor_scalar(
                            out=mean, in0=s1, scalar1=1.0 / D, scalar2=None,
                            op0=mybir.AluOpType.mult,
                        )
                        msq = sc1.tile([P, 1], F32, tag="msq", name="msq")
                        nc.vector.tensor_mul(msq, mean, mean)
                        var = sc1.tile([P, 1], F32, tag="var", name="var")
                        nc.vector.scalar_tensor_tensor(
                            out=var, in0=s2, scalar=1.0 / D, in1=msq,
                            op0=mybir.AluOpType.mult, op1=mybir.AluOpType.subtract,
                        )
                        std = sc1.tile([P, 1], F32, tag="std", name="std")
                        nc.scalar.activation(
                            out=std, in_=var,
                            func=mybir.ActivationFunctionType.Sqrt, bias=eps_t,
                        )
                        rstd = sc1.tile([P, 1], F32, tag="rstd", name="rstd")
                        nc.vector.reciprocal(rstd, std)

                        xn_s = xnp.tile([P, D], F16, tag="xn", name="xn_s")
                        nc.gpsimd.tensor_scalar(
                            out=xn_s, in0=x_s, scalar1=mean, scalar2=rstd,
                            op0=mybir.AluOpType.subtract, op1=mybir.AluOpType.mult,
                        )
                        nc.sync.dma_start_transpose(xnT_all[:, t, :, :], xn_s)

                    # ---- proj pass with interleaved (qb0, pair 0/1) chunks ----
                    new_attn(0)
                    next_kb = [0] * NPAIR

                    def proj(t):
                        r0 = t * P
                        rot = rotp.tile([P, 2 * IN], F16, tag="rot", name="rot")
                        ta_sl = ta_s[:, t, :]
                        tb_sl = tb_s[:, t, :]
                        ta_b = bass.AP(tensor=ta_sl.tensor, offset=ta_sl.offset,
                                       ap=[list(ta_sl.ap[0]), [0, NH], list(ta_sl.ap[-1])])
                        tb_b = bass.AP(tensor=tb_sl.tensor, offset=tb_sl.offset,
                                       ap=[list(tb_sl.ap[0]), [0, NH], list(tb_sl.ap[-1])])
                        for i, w_s in enumerate((wq_s, wk_s, wv_s)):
                            # projections ride the score-psum ring (bank 0 of a slot)
                            pslot = pst.tile([P, 2 * QB], F32, tag="st", name="ps_proj")
                            ps = pslot[:, 0:IN]
                            for c in range(NCK):
                                nc.tensor.matmul(
                                    ps, lhsT=xnT_all[:, t, c, :], rhs=w_s[:, c, :],
                                    start=(c == 0), stop=(c == NCK - 1),
                                    skip_group_check=True,
                                )
                            if i == 2:
                                nc.vector.tensor_copy(
                                    out=v_s[:, t, :, 0:DH],
                                    in_=ps.rearrange("p (h d) -> p h d", h=NH),
                                )
                            else:
                                p3 = ps.rearrange("p (h d) -> p h d", h=NH)
                                a_t = abp.tile([P, NH, DH], F16, tag="a", name="a_t")
                                b_t = abp.tile([P, NH, DH], F16, tag="b", name="b_t")
                                nc.vector.tensor_mul(a_t, p3, ta_b)
                                nc.vector.tensor_mul(b_t, p3, tb_b)
                                r3 = rot[:, i * IN : (i + 1) * IN].rearrange(
                                    "p (h d) -> p h d", h=NH
                                )
                                nc.vector.tensor_sub(
                                    r3[:, :, 0:32], a_t[:, :, 0:32], a_t[:, :, 32:64]
                                )
                                nc.vector.tensor_add(
                                    r3[:, :, 32:64], b_t[:, :, 0:32], b_t[:, :, 32:64]
                                )
                        nc.sync.dma_start_transpose(qkT[:, :, r0 : r0 + P], rot)

                    for t in range(NT):
                        proj(t)
                        if t >= 3:
                            for pair in (0, 1):
                                while next_kb[pair] <= t - 2:
                                    chunk(0, pair, next_kb[pair], pv_of[pair])
                                    next_kb[pair] += 1

                    # finish qb0 under epoch-1 pools (pv_extra must stay open)
                    for pair in (0, 1):
                        for kb in range(next_kb[pair], NT):
                            chunk(0, pair, kb, pv_of[pair])
                        pair_epilogue(0, pair, pv_of[pair])
                    for pair in (2, 3):
                        for kb in range(NT):
                            chunk(0, pair, kb, pv_of[pair])
                        pair_epilogue(0, pair, pv_of[pair])

                # ============ epoch 2: qb1..3 + out-projections ============
                with tc.tile_pool(name="pso", bufs=2, space="PSUM") as pso, \
                     tc.tile_pool(name="osb", bufs=3) as osb:
                    qb_outproj(0, pso, osb)
                    for qb in range(1, NQB):
                        new_attn(qb)
                        for pair in range(NPAIR):
                            for kb in range(NT):
                                chunk(qb, pair, kb, pv_main)
                            pair_epilogue(qb, pair, pv_main)
                        qb_outproj(qb, pso, osb)
    nc.compile()
    return nc


def _rope_tables():
    inv = 1.0 / (BASE ** (np.arange(0, DH, 2, dtype=np.float32) / DH))
    t = np.arange(S, dtype=np.float32)
    freqs = t[:, None] * inv[None, :]  # [S, 32]
    c, s = np.cos(freqs), np.sin(freqs)
    tab_a = np.concatenate([c, s], axis=1).astype(np.float16)  # [S, 64]
    tab_b = np.concatenate([s, c], axis=1).astype(np.float16)
    return np.ascontiguousarray(tab_a), np.ascontiguousarray(tab_b)


def kernel(x, w_qkv, w_out, b_out, ln_gamma, ln_beta, _want_results=False, _trace=False):
    x = np.asarray(x, dtype=np.float32)
    w_qkv = np.asarray(w_qkv, dtype=np.float32)
    w_out = np.asarray(w_out, dtype=np.float32)
    b_out = np.asarray(b_out, dtype=np.float32)
    ln_gamma = np.asarray(ln_gamma, dtype=np.float32)
    ln_beta = np.asarray(ln_beta, dtype=np.float32)
    assert np.all(ln_beta == 0.0), "nonzero ln_beta not supported by this kernel"

    if "nc" not in _CACHE:
        _CACHE["nc"] = _build_nc()
    nc = _CACHE["nc"]

    wg = w_qkv * ln_gamma[:, None]  # fold gamma into the projection
    tab_a, tab_b = _rope_tables()

    in_maps = []
    for core in range(8):
        b, hg = core // HG, core % HG
        c0 = hg * IN
        in_maps.append({
            "x": np.ascontiguousarray(x[b]),
            "wq": np.ascontiguousarray(wg[:, c0 : c0 + IN]).astype(np.float16),
            "wk": np.ascontiguousarray(wg[:, D + c0 : D + c0 + IN]).astype(np.float16),
            "wv": np.ascontiguousarray(wg[:, 2 * D + c0 : 2 * D + c0 + IN]).astype(np.float16),
            "wo": np.ascontiguousarray(w_out[c0 : c0 + IN, :]).astype(np.float16),
            "tab_a": tab_a,
            "tab_b": tab_b,
        })

    res = run_bass_kernel_spmd(nc, in_maps, list(range(8)), trace=_trace)
    parts = [np.asarray(res.results[c]["out"]) for c in range(8)]
    out = np.empty((B, S, D), dtype=np.float32)
    for b in range(B):
        out[b] = parts[2 * b].astype(np.float32) + parts[2 * b + 1].astype(np.float32) + b_out[None, :]
    if _want_results:
        return out, res
    return out


if __name__ == "__main__":
    rng = np.random.default_rng(0)
    inputs = {
        "x": rng.standard_normal((B, S, D), dtype=np.float32),
        "w_qkv": (rng.standard_normal((D, 3 * D), dtype=np.float32) * D ** -0.5),
        "w_out": (rng.standard_normal((D, D), dtype=np.float32) * D ** -0.5),
        "b_out": np.zeros(D, np.float32),
        "ln_gamma": np.ones(D, np.float32),
        "ln_beta": np.zeros(D, np.float32),
    }
    out = kernel(**inputs)
    print("ok", out.shape, out.dtype)


# revision 13
# speedup vs baseline: 2.0968x; 1.0040x over previous
"""Trainium2 Bass kernel for nn_Attention_RoPE (LN -> QKV -> RoPE -> attention -> out-proj).

Sharding: 8 cores = 4 batches x 2 head-groups (8 heads each).
Each core computes a partial out-projection [S, D] (fp16) for its
(batch, head-group); host sums the two partials per batch and adds b_out.

Per-core pipeline (single Bass program, SPMD over 8 cores), fp16 matmuls:
  LN pass (per 128-row seq tile): stats via ScalarE accum (Copy/Square +
    accum_out), normalize on GPSIMD -> fp16, xn -> xnT via DMA xbar
    transpose. All sqrts precede all exps (single act-table switch).
  proj pass (per tile): QKV matmuls (fp16), RoPE on q,k (DVE: two
    elementwise products against [cos|sin]/[sin|cos] tables broadcast
    per head + add/sub), one DMA xbar transpose -> qkT; v -> SBUF fp16
    with an appended ones column (softmax denominators ride the PV
    matmul). Attention chunks for (qb0, pairs 0/1) are interleaved
    between tiles to keep the PE ramped and start the exp stream early.
  attention chunk (qb, pair, kb): S^T = K@Q^T (two PE-quadrant matmuls,
    kpos on partitions, K=64) -> exp on ScalarE [128,1024] -> PV with
    moving dim = dh+1 (N=65, q on output partitions) accumulating over
    kb. The four 128-query sub-blocks share one PSUM bank per head:
    matmul start=True zeroes the whole bank, so only the first matmul
    per (pair, bank) round sets it (skip_group_check).
  epilogue per (qb,pair): DVE reciprocal + per-partition tensor_scalar
    scale -> attn_sb fp16; per qb: DMA xbar transpose -> attnT,
    out-projection, fp16 partial DMA'd out.
"""

import numpy as np
import sys

sys.path.insert(0, "/opt/trn_rl_repo")

import concourse.bass as bass
from concourse import bacc
import concourse.mybir as mybir
import concourse.tile as tile
from concourse.bass_utils import run_bass_kernel_spmd

# Problem constants (hardcoded per contract)
B, S, D = 4, 2048, 1024
H, DH = 16, 64
HG = 2              # head groups (tensor-parallel dim)
NH = H // HG        # heads per core = 8
IN = NH * DH        # per-core inner dim = 512
P = 128
NT = S // P         # 16 seq tiles
NCK = D // P        # 8 contraction chunks
NPAIR = NH // 2     # 4 head pairs
QB = 512            # query block
NQB = S // QB       # 4 query blocks
EPS = 1e-5
BASE = 10000.0

F32 = mybir.dt.float32
F16 = mybir.dt.float16

_CACHE = {}


def _build_nc():
    nc = bacc.Bacc(None, target_bir_lowering=False, debug=False)

    x_d = nc.declare_dram_parameter("x", [S, D], F32, isOutput=False)
    wq_d = nc.declare_dram_parameter("wq", [D, IN], F16, isOutput=False)
    wk_d = nc.declare_dram_parameter("wk", [D, IN], F16, isOutput=False)
    wv_d = nc.declare_dram_parameter("wv", [D, IN], F16, isOutput=False)
    wo_d = nc.declare_dram_parameter("wo", [IN, D], F16, isOutput=False)
    ta_d = nc.declare_dram_parameter("tab_a", [S, DH], F16, isOutput=False)
    tb_d = nc.declare_dram_parameter("tab_b", [S, DH], F16, isOutput=False)
    out_d = nc.declare_dram_parameter("out", [S, D], F16, isOutput=True)

    scale = 1.0 / np.sqrt(DH)

    with tile.TileContext(nc) as tc:
        with tc.tile_pool(name="persist", bufs=1) as pers:
            eps_t = pers.tile([P, 1], F32)
            nc.vector.memset(eps_t, EPS)

            # fp16 weights resident in SBUF; row (c, p) <-> contraction d = c*128+p
            wq_s = pers.tile([P, NCK, IN], F16, tag="wq")
            wk_s = pers.tile([P, NCK, IN], F16, tag="wk")
            wv_s = pers.tile([P, NCK, IN], F16, tag="wv")
            wo_s = pers.tile([P, 4, D], F16, tag="wo")
            # RoPE tables [seq tile, 64]: A = [cos|sin], B = [sin|cos]
            ta_s = pers.tile([P, NT, DH], F16, tag="ta")
            tb_s = pers.tile([P, NT, DH], F16, tag="tb")

            # persistent activations
            # qkT: g<4 -> q pair g; g>=4 -> k pair g-4; row p = hh*64 + d
            qkT = pers.tile([P, 2 * NPAIR, S], F16, tag="qkT")
            v_s = pers.tile([P, NT, NH, DH + 1], F16, tag="v")
            nc.vector.memset(v_s[:, :, :, DH : DH + 1], 1.0)
            xnT_all = pers.tile([P, NT, NCK, P], F16, tag="xnT")

            nc.sync.dma_start(out=ta_s, in_=ta_d.rearrange("(t p) d -> p t d", p=P))
            nc.sync.dma_start(out=tb_s, in_=tb_d.rearrange("(t p) d -> p t d", p=P))
            for w_s, w_d, ncw in ((wq_s, wq_d, NCK), (wk_s, wk_d, NCK),
                                  (wv_s, wv_d, NCK), (wo_s, wo_d, 4)):
                for c in range(ncw):
                    nc.sync.dma_start(out=w_s[:, c, :], in_=w_d[c * P : (c + 1) * P, :])

            with tc.tile_pool(name="pst", bufs=2, space="PSUM") as pst, \
                 tc.tile_pool(name="pvm", bufs=1, space="PSUM") as pvm, \
                 tc.tile_pool(name="ptp", bufs=3) as ptp, \
                 tc.tile_pool(name="sc3", bufs=4) as sc3, \
                 tc.tile_pool(name="attn", bufs=2) as atp, \
                 tc.tile_pool(name="attnT", bufs=2) as atpT:

                # PV accumulators (pv slot = 2 banks): [q, qs, d*72pad|den@64]
                pv_main = [pvm.tile([P, 4, 72], F32, tag=f"pvm{hh}", name=f"pvm{hh}")
                           for hh in range(2)]

                attn_sb = {}   # qb -> [q, qs, feat] fp16

                def chunk(qb, pair, kb, pv):
                    """scores + exp + PV(+denominator column) for one (qb, pair, kb)."""
                    q0 = qb * QB
                    k0 = kb * P
                    ps_st = pst.tile([P, 2 * QB], F32, tag="st", name="ps_st")
                    for hh in range(2):
                        f0 = hh * DH
                        nc.tensor.matmul(
                            ps_st[:, hh * QB : (hh + 1) * QB],
                            lhsT=qkT[f0 : f0 + DH, NPAIR + pair, k0 : k0 + P],
                            rhs=qkT[f0 : f0 + DH, pair, q0 : q0 + QB],
                            start=True, stop=True,
                        )
                    pt = ptp.tile([P, 2 * QB], F16, tag="pt", name="pt")
                    nc.scalar.activation(
                        out=pt, in_=ps_st,
                        func=mybir.ActivationFunctionType.Exp, scale=scale,
                    )
                    for hh in range(2):
                        for qs in range(4):
                            nc.tensor.matmul(
                                pv[hh][:, qs, 0 : DH + 1],
                                lhsT=pt[:, hh * QB + qs * P : hh * QB + (qs + 1) * P],
                                rhs=v_s[:, kb, 2 * pair + hh, :],
                                start=(kb == 0 and qs == 0),
                                stop=(kb == NT - 1 and qs == 3),
                                skip_group_check=True,
                            )

                def pair_epilogue(qb, pair, pv, per_qs=None):
                    a_sb = attn_sb[qb]
                    recs = []
                    for hh in range(2):
                        rec = sc3.tile([P, 4, 1], F32, tag="rec", name="rec")
                        nc.vector.reciprocal(rec, pv[hh][:, :, DH : DH + 1])
                        recs.append(rec)
                    for qs in range(4):
                        for hh in range(2):
                            nc.vector.tensor_scalar(
                                out=a_sb[:, qs, pair * P + hh * DH : pair * P + (hh + 1) * DH],
                                in0=pv[hh][:, qs, 0:DH],
                                scalar1=recs[hh][:, qs, :], scalar2=None,
                                op0=mybir.AluOpType.mult,
                            )
                        if per_qs is not None:
                            per_qs(qs)

                def new_attn(qb):
                    attn_sb[qb] = atp.tile([P, 4, IN], F16, tag="attn", name=f"attn{qb}")

                def qb_outproj_steps(qb, pso, osb):
                    """Emit out-projection for qb as a list of step closures so
                    the caller can interleave them with the next qb's chunks
                    (keeps the PE fed without starving the ScalarE exp stream)."""
                    aT = atpT.tile([P, 4, QB], F16, tag="aT", name=f"aT{qb}")
                    a_sb = attn_sb[qb]
                    steps = []

                    def dmaT(qs):
                        nc.sync.dma_start_transpose(
                            aT[:, :, qs * P : (qs + 1) * P], a_sb[:, qs, :]
                        )

                    def make_group(qs, n, o_sb):
                        def f():
                            ps_o = pso.tile([P, 512], F32, tag="po", name="ps_o")
                            for c in range(4):
                                nc.tensor.matmul(
                                    ps_o,
                                    lhsT=aT[:, c, qs * P : (qs + 1) * P],
                                    rhs=wo_s[:, c, n * 512 : (n + 1) * 512],
                                    start=(c == 0), stop=(c == 3),
                                )
                            nc.vector.tensor_copy(
                                out=o_sb[:, n * 512 : (n + 1) * 512], in_=ps_o
                            )
                            if n == 1:
                                r0 = qb * QB + qs * P
                                nc.scalar.dma_start(out=out_d[r0 : r0 + P, :], in_=o_sb)
                        return f

                    steps.append(lambda: [dmaT(qs) for qs in range(4)])
                    for qs in range(4):
                        o_sb = osb.tile([P, D], F16, tag="osb", name="o_sb")
                        for n in range(2):
                            steps.append(make_group(qs, n, o_sb))
                    return steps

                # ============ epoch 1: LN pass, proj pass + qb0 interleave ====
                with tc.tile_pool(name="pve", bufs=1, space="PSUM") as pve, \
                     tc.tile_pool(name="xp", bufs=3) as xp, \
                     tc.tile_pool(name="sc1", bufs=4) as sc1, \
                     tc.tile_pool(name="xnp", bufs=3) as xnp, \
                     tc.tile_pool(name="rotp", bufs=3) as rotp, \
                     tc.tile_pool(name="abp", bufs=2) as abp:

                    pv_extra = [pve.tile([P, 4, 72], F32, tag=f"pve{hh}", name=f"pve{hh}")
                                for hh in range(2)]
                    pv_of = {0: pv_main, 1: pv_extra, 2: pv_main, 3: pv_extra}

                    # ---- LN pass: stats on DVE; x DMAs prefetched ahead of the
                    # normalize ops that share the in-order Pool queue ----
                    x_tiles = []

                    def emit_xdma(t):
                        x_s = xp.tile([P, D], F32, tag="x", name="x_s")
                        nc.gpsimd.dma_start(out=x_s, in_=x_d[t * P : (t + 1) * P, :])
                        x_tiles.append(x_s)

                    for t in range(3):
                        emit_xdma(t)
                    for t in range(NT):
                        x_s = x_tiles[t]
                        stats = sc1.tile([P, 2, 6], F32, tag="stats", name="stats")
                        for i in range(2):
                            nc.vector.bn_stats(
                                out=stats[:, i, :], in_=x_s[:, i * 512 : (i + 1) * 512]
                            )
                        mv = sc1.tile([P, 2], F32, tag="mv", name="mv")
                        nc.vector.bn_aggr(out=mv, in_=stats)
                        std = sc1.tile([P, 1], F32, tag="std", name="std")
                        nc.scalar.activation(
                            out=std, in_=mv[:, 1:2],
                            func=mybir.ActivationFunctionType.Sqrt, bias=eps_t,
                        )
                        rstd = sc1.tile([P, 1], F32, tag="rstd", name="rstd")
                        nc.vector.reciprocal(rstd, std)

                        xn_s = xnp.tile([P, D], F16, tag="xn", name="xn_s")
                        nc.gpsimd.tensor_scalar(
                            out=xn_s, in0=x_s, scalar1=mv[:, 0:1], scalar2=rstd,
                            op0=mybir.AluOpType.subtract, op1=mybir.AluOpType.mult,
                        )
                        if t + 3 < NT:
                            emit_xdma(t + 3)
                        nc.sync.dma_start_transpose(xnT_all[:, t, :, :], xn_s)

                    # ---- proj pass with interleaved (qb0, pair 0/1) chunks ----
                    new_attn(0)
                    next_kb = [0] * NPAIR

                    def proj(t):
                        r0 = t * P
                        rot = rotp.tile([P, 2 * IN], F16, tag="rot", name="rot")
                        ta_sl = ta_s[:, t, :]
                        tb_sl = tb_s[:, t, :]
                        ta_b = bass.AP(tensor=ta_sl.tensor, offset=ta_sl.offset,
                                       ap=[list(ta_sl.ap[0]), [0, NH], list(ta_sl.ap[-1])])
                        tb_b = bass.AP(tensor=tb_sl.tensor, offset=tb_sl.offset,
                                       ap=[list(tb_sl.ap[0]), [0, NH], list(tb_sl.ap[-1])])
                        for i, w_s in enumerate((wq_s, wk_s, wv_s)):
                            # projections ride the score-psum ring (bank 0 of a slot)
                            pslot = pst.tile([P, 2 * QB], F32, tag="st", name="ps_proj")
                            ps = pslot[:, 0:IN]
                            for c in range(NCK):
                                nc.tensor.matmul(
                                    ps, lhsT=xnT_all[:, t, c, :], rhs=w_s[:, c, :],
                                    start=(c == 0), stop=(c == NCK - 1),
                                    skip_group_check=True,
                                )
                            if i == 2:
                                nc.vector.tensor_copy(
                                    out=v_s[:, t, :, 0:DH],
                                    in_=ps.rearrange("p (h d) -> p h d", h=NH),
                                )
                            else:
                                p3 = ps.rearrange("p (h d) -> p h d", h=NH)
                                a_t = abp.tile([P, NH, DH], F16, tag="a", name="a_t")
                                b_t = abp.tile([P, NH, DH], F16, tag="b", name="b_t")
                                nc.vector.tensor_mul(a_t, p3, ta_b)
                                nc.vector.tensor_mul(b_t, p3, tb_b)
                                r3 = rot[:, i * IN : (i + 1) * IN].rearrange(
                                    "p (h d) -> p h d", h=NH
                                )
                                nc.vector.tensor_sub(
                                    r3[:, :, 0:32], a_t[:, :, 0:32], a_t[:, :, 32:64]
                                )
                                nc.vector.tensor_add(
                                    r3[:, :, 32:64], b_t[:, :, 0:32], b_t[:, :, 32:64]
                                )
                        nc.sync.dma_start_transpose(qkT[:, :, r0 : r0 + P], rot)

                    for t in range(NT):
                        proj(t)
                        if t >= 3:
                            for pair in (0, 1):
                                while next_kb[pair] <= t - 2:
                                    chunk(0, pair, next_kb[pair], pv_of[pair])
                                    next_kb[pair] += 1

                    # finish qb0 under epoch-1 pools (pv_extra must stay open)
                    for pair in (0, 1):
                        for kb in range(next_kb[pair], NT):
                            chunk(0, pair, kb, pv_of[pair])
                        pair_epilogue(0, pair, pv_of[pair])
                    for pair in (2, 3):
                        for kb in range(NT):
                            chunk(0, pair, kb, pv_of[pair])
                        pair_epilogue(0, pair, pv_of[pair])

                # ============ epoch 2: qb1..3 + out-projections ============
                with tc.tile_pool(name="pso", bufs=2, space="PSUM") as pso, \
                     tc.tile_pool(name="osb", bufs=3) as osb:
                    pending = qb_outproj_steps(0, pso, osb)
                    for qb in range(1, NQB):
                        new_attn(qb)
                        last_qb = qb == NQB - 1
                        for pair in range(NPAIR):
                            for kb in range(NT):
                                chunk(qb, pair, kb, pv_main)
                                if pending:
                                    pending.pop(0)()
                            if last_qb and pair == NPAIR - 1:
                                # drain the tail: out-project each 128-query
                                # sub-block right after its final scale
                                aT = atpT.tile([P, 4, QB], F16, tag="aT", name="aT3")
                                a_sb = attn_sb[qb]

                                def per_qs(qs):
                                    nc.sync.dma_start_transpose(
                                        aT[:, :, qs * P : (qs + 1) * P], a_sb[:, qs, :]
                                    )
                                    o_sb = osb.tile([P, D], F16, tag="osb", name="o_sb")
                                    for n in range(2):
                                        ps_o = pso.tile([P, 512], F32, tag="po", name="ps_o")
                                        for c in range(4):
                                            nc.tensor.matmul(
                                                ps_o,
                                                lhsT=aT[:, c, qs * P : (qs + 1) * P],
                                                rhs=wo_s[:, c, n * 512 : (n + 1) * 512],
                                                start=(c == 0), stop=(c == 3),
                                            )
                                        nc.vector.tensor_copy(
                                            out=o_sb[:, n * 512 : (n + 1) * 512], in_=ps_o
                                        )
                                    r0 = qb * QB + qs * P
                                    nc.scalar.dma_start(out=out_d[r0 : r0 + P, :], in_=o_sb)

                                pair_epilogue(qb, pair, pv_main, per_qs=per_qs)
                            else:
                                pair_epilogue(qb, pair, pv_main)
                        if not last_qb:
                            pending = qb_outproj_steps(qb, pso, osb)
    nc.compile()
    return nc


def _rope_tables():
    inv = 1.0 / (BASE ** (np.arange(0, DH, 2, dtype=np.float32) / DH))
    t = np.arange(S, dtype=np.float32)
    freqs = t[:, None] * inv[None, :]  # [S, 32]
    c, s = np.cos(freqs), np.sin(freqs)
    tab_a = np.concatenate([c, s], axis=1).astype(np.float16)  # [S, 64]
    tab_b = np.concatenate([s, c], axis=1).astype(np.float16)
    return np.ascontiguousarray(tab_a), np.ascontiguousarray(tab_b)


def kernel(x, w_qkv, w_out, b_out, ln_gamma, ln_beta, _want_results=False, _trace=False):
    x = np.asarray(x, dtype=np.float32)
    w_qkv = np.asarray(w_qkv, dtype=np.float32)
    w_out = np.asarray(w_out, dtype=np.float32)
    b_out = np.asarray(b_out, dtype=np.float32)
    ln_gamma = np.asarray(ln_gamma, dtype=np.float32)
    ln_beta = np.asarray(ln_beta, dtype=np.float32)
    assert np.all(ln_beta == 0.0), "nonzero ln_beta not supported by this kernel"

    if "nc" not in _CACHE:
        _CACHE["nc"] = _build_nc()
    nc = _CACHE["nc"]

    wg = w_qkv * ln_gamma[:, None]  # fold gamma into the projection
    tab_a, tab_b = _rope_tables()

    in_maps = []
    for core in range(8):
        b, hg = core // HG, core % HG
        c0 = hg * IN
        in_maps.append({
            "x": np.ascontiguousarray(x[b]),
            "wq": np.ascontiguousarray(wg[:, c0 : c0 + IN]).astype(np.float16),
            "wk": np.ascontiguousarray(wg[:, D + c0 : D + c0 + IN]).astype(np.float16),
            "wv": np.ascontiguousarray(wg[:, 2 * D + c0 : 2 * D + c0 + IN]).astype(np.float16),
            "wo": np.ascontiguousarray(w_out[c0 : c0 + IN, :]).astype(np.float16),
            "tab_a": tab_a,
            "tab_b": tab_b,
        })

    res = run_bass_kernel_spmd(nc, in_maps, list(range(8)), trace=_trace)
    parts = [np.asarray(res.results[c]["out"]) for c in range(8)]
    out = np.empty((B, S, D), dtype=np.float32)
    for b in range(B):
        out[b] = parts[2 * b].astype(np.float32) + parts[2 * b + 1].astype(np.float32) + b_out[None, :]
    if _want_results:
        return out, res
    return out


if __name__ == "__main__":
    rng = np.random.default_rng(0)
    inputs = {
        "x": rng.standard_normal((B, S, D), dtype=np.float32),
        "w_qkv": (rng.standard_normal((D, 3 * D), dtype=np.float32) * D ** -0.5),
        "w_out": (rng.standard_normal((D, D), dtype=np.float32) * D ** -0.5),
        "b_out": np.zeros(D, np.float32),
        "ln_gamma": np.ones(D, np.float32),
        "ln_beta": np.zeros(D, np.float32),
    }
    out = kernel(**inputs)
    print("ok", out.shape, out.dtype)
